# revision 1
# baseline (speedup 1.0000x reference)
"""RNN-T Joiner kernel for Trainium2, data-parallel over (B, T) on 8 cores.

reference:
    logit = tanh(enc[:, :, None, :] + dec[:, None, :, :])   # (B,T,U,C)
    out   = einsum('btuc,vc->btuv', logit, W) + b           # (B,T,U,V)

Shapes (hardcoded): B=4, T=256, U=64, C=512, V=1024.

Sharding: core k handles b = k//2, t rows [ (k%2)*128, (k%2)*128+128 ).
W / bias replicated. No collectives.

Per-core device kernel (C on partitions for the logit):
  - logitT[c, t] = tanh(encT[c, t] + decT[c, u])  -- scalar engine, fused
    per-partition bias add.
  - out[t, v] accumulated over 4 c-chunks of K=128 matmuls; inputs bitcast
    to float32r (full PE rate at out-free-dim >= 256, fp32 data).
  - bias add fused into the PSUM->SBUF eviction on DVE.
  - out tile DMA'd straight to DRAM (2KB contiguous per partition).
"""

import numpy as np

B, T, U, C, V = 4, 256, 64, 512, 1024
NCORES = 8
TS = 128  # t rows per core
CCH = C // 128  # 4 contraction chunks
VH = V // 512  # 2 psum-width chunks

_CACHE = {}


def _build():
    from contextlib import ExitStack

    import concourse.bacc as bacc
    import concourse.mybir as mybir
    import concourse.tile as tile

    dt = mybir.dt
    f32 = dt.float32
    f32r = dt.float32r

    nc = bacc.Bacc("TRN2", target_bir_lowering=False, debug=False, num_devices=NCORES)
    enc_t = nc.declare_dram_parameter("enc_t", [C, TS], f32, isOutput=False)
    dec_t = nc.declare_dram_parameter("dec_t", [C, U], f32, isOutput=False)
    wt = nc.declare_dram_parameter("wt", [C, V], f32r, isOutput=False)
    bias_rep = nc.declare_dram_parameter("bias_rep", [128, V], f32, isOutput=False)
    out = nc.declare_dram_parameter("out", [TS, U, V], f32, isOutput=True)

    with tile.TileContext(nc) as tc, ExitStack() as ctx:
        const = ctx.enter_context(tc.tile_pool(name="const", bufs=1))
        logit_pool = ctx.enter_context(tc.tile_pool(name="logit", bufs=6))
        psum_pool = ctx.enter_context(tc.tile_pool(name="psum", bufs=4, space="PSUM"))
        out_pool = ctx.enter_context(tc.tile_pool(name="out", bufs=6))

        wt_sb = const.tile([128, CCH * V], f32r, tag="wt")
        enc_sb = const.tile([128, CCH * TS], f32, tag="enc")
        dec_sb = const.tile([128, CCH * U], f32, tag="dec")
        bias_sb = const.tile([128, V], f32, tag="bias")

        nc.sync.dma_start(
            enc_sb[:].rearrange("p (c t) -> p c t", c=CCH),
            enc_t[:].rearrange("(c p) t -> p c t", p=128),
        )
        nc.sync.dma_start(
            dec_sb[:].rearrange("p (c u) -> p c u", c=CCH),
            dec_t[:].rearrange("(c p) u -> p c u", p=128),
        )
        for c in range(CCH):
            nc.sync.dma_start(
                wt_sb[:, c * V : (c + 1) * V], wt[c * 128 : (c + 1) * 128, :]
            )
        nc.sync.dma_start(bias_sb[:], bias_rep[:])

        for u in range(U):
            lg = logit_pool.tile([128, CCH * TS], f32r, tag="lg")
            for c in range(CCH):
                nc.scalar.activation(
                    lg[:, c * TS : (c + 1) * TS],
                    enc_sb[:, c * TS : (c + 1) * TS],
                    mybir.ActivationFunctionType.Tanh,
                    bias=dec_sb[:, c * U + u : c * U + u + 1],
                )
            ps = psum_pool.tile([128, V], f32, tag="ps")
            for vh in range(VH):
                for c in range(CCH):
                    nc.tensor.matmul(
                        ps[:, vh * 512 : (vh + 1) * 512],
                        lhsT=lg[:, c * TS : (c + 1) * TS],
                        rhs=wt_sb[:, c * V + vh * 512 : c * V + vh * 512 + 512],
                        start=(c == 0),
                        stop=(c == CCH - 1),
                    )
            ob = out_pool.tile([128, V], f32, tag="ob")
            nc.vector.tensor_add(ob[:], ps[:], bias_sb[:])
            nc.sync.dma_start(out[:, u, :], ob[:])

    nc.finalize()
    return nc


def _get_nc():
    if "nc" not in _CACHE:
        _CACHE["nc"] = _build()
    return _CACHE["nc"]


def kernel(**inputs):
    enc = np.asarray(inputs["enc_out"], dtype=np.float32)
    dec = np.asarray(inputs["dec_out"], dtype=np.float32)
    W = np.asarray(inputs["W"], dtype=np.float32)
    b = np.asarray(inputs["b"], dtype=np.float32)

    nc = _get_nc()

    wt_np = np.ascontiguousarray(W.T)
    bias_np = np.ascontiguousarray(np.broadcast_to(b, (128, V)))
    in_maps = []
    for k in range(NCORES):
        bb, t0 = k // 2, (k % 2) * TS
        in_maps.append(
            {
                "enc_t": np.ascontiguousarray(enc[bb, t0 : t0 + TS, :].T),
                "dec_t": np.ascontiguousarray(dec[bb].T),
                "wt": wt_np,
                "bias_rep": bias_np,
            }
        )

    from concourse.bass_utils import run_bass_kernel_spmd

    res = run_bass_kernel_spmd(nc, in_maps, list(range(NCORES)))
    _CACHE["last_result"] = res

    out = np.empty((B, T, U, V), np.float32)
    for k in range(NCORES):
        bb, t0 = k // 2, (k % 2) * TS
        out[bb, t0 : t0 + TS] = res.results[k]["out"]
    return out



# revision 3
# speedup vs baseline: 1.0198x; 1.0198x over previous
"""RNN-T Joiner kernel for Trainium2, data-parallel over (B, T) on 8 cores.

reference:
    logit = tanh(enc[:, :, None, :] + dec[:, None, :, :])   # (B,T,U,C)
    out   = einsum('btuc,vc->btuv', logit, W) + b           # (B,T,U,V)

Shapes (hardcoded): B=4, T=256, U=64, C=512, V=1024.

Sharding: core k handles b = k//2, t rows [ (k%2)*128, (k%2)*128+128 ).
W / bias replicated. No collectives.

Per-core device kernel (C on partitions for the logit):
  - logitT[c, t] = tanh(encT[c, t] + decT[c, u]) in bf16 -- scalar engine,
    fused per-partition bias add.
  - out[t, v] accumulated over 4 c-chunks of K=128 matmuls; both operands
    bf16 (full PE stream rate, fast weight load).
  - W / bias pre-cast to bf16 on host; output written bf16, upcast on host.
  - warmup matmuls on a memset tile run during the input DMA window so the
    PE HAM clock-gate is released before the real matmuls start.
  - bias add fused into the PSUM->SBUF eviction on DVE; 4 u-steps batched
    per output DMA (8KB contiguous per partition).
"""

import numpy as np

B, T, U, C, V = 4, 256, 64, 512, 1024
NCORES = 8
TS = 128  # t rows per core
CCH = C // 128  # 4 contraction chunks
VH = V // 512  # 2 psum-width chunks
UG = 4  # u-steps per output DMA batch
NWARM = 7  # warmup matmuls (N=512) to release the PE clock gate

_CACHE = {}


def _build():
    from contextlib import ExitStack

    import concourse.bacc as bacc
    import concourse.mybir as mybir
    import concourse.tile as tile

    dt = mybir.dt
    f32 = dt.float32
    bf16 = dt.bfloat16

    nc = bacc.Bacc("TRN2", target_bir_lowering=False, debug=False, num_devices=NCORES)
    enc_t = nc.declare_dram_parameter("enc_t", [C, TS], f32, isOutput=False)
    dec_t = nc.declare_dram_parameter("dec_t", [C, U], f32, isOutput=False)
    wt = nc.declare_dram_parameter("wt", [C, V], bf16, isOutput=False)
    bias_rep = nc.declare_dram_parameter("bias_rep", [128, V], bf16, isOutput=False)
    out = nc.declare_dram_parameter("out", [TS, U, V], bf16, isOutput=True)

    with tile.TileContext(nc) as tc, ExitStack() as ctx:
        const = ctx.enter_context(tc.tile_pool(name="const", bufs=1))
        logit_pool = ctx.enter_context(tc.tile_pool(name="logit", bufs=6))
        psum_pool = ctx.enter_context(tc.tile_pool(name="psum", bufs=4, space="PSUM"))
        out_pool = ctx.enter_context(tc.tile_pool(name="out", bufs=3))

        wt_sb = const.tile([128, CCH * V], bf16, tag="wt")
        enc_sb = const.tile([128, CCH * TS], f32, tag="enc")
        dec_sb = const.tile([128, CCH * U], f32, tag="dec")
        bias_sb = const.tile([128, V], bf16, tag="bias")
        dummy = const.tile([128, 640], bf16, tag="dummy")

        # Warmup source: memset on gpsimd (idle at startup), no DMA dep.
        nc.gpsimd.memset(dummy[:], 0.0)

        # Input DMAs: spread triggers over both HWDGE rings so they all
        # issue within ~1.3us of the preamble ending.
        nc.scalar.dma_start(
            enc_sb[:].rearrange("p (c t) -> p c t", c=CCH),
            enc_t[:].rearrange("(c p) t -> p c t", p=128),
        )
        nc.scalar.dma_start(
            dec_sb[:].rearrange("p (c u) -> p c u", c=CCH),
            dec_t[:].rearrange("(c p) u -> p c u", p=128),
        )
        nc.sync.dma_start(
            wt_sb[:].rearrange("p (c v) -> p c v", c=CCH),
            wt[:].rearrange("(c p) v -> p c v", p=128),
        )
        nc.sync.dma_start(bias_sb[:], bias_rep[:])

        # Warmup matmuls: keep the PE busy while inputs stream in, so the
        # HAM clock-gate (4/8 cold -> 8/8 warm after ~3.4us of activity)
        # opens before the first real matmul.
        psw = psum_pool.tile([128, V], f32, tag="ps")
        for _ in range(NWARM):
            nc.tensor.matmul(
                psw[:, 0:512],
                lhsT=dummy[:, 0:128],
                rhs=dummy[:, 128:640],
                start=True,
                stop=True,
            )

        ob = None
        for u in range(U):
            lg = logit_pool.tile([128, CCH * TS], bf16, tag="lg")
            for c in range(CCH):
                nc.scalar.activation(
                    lg[:, c * TS : (c + 1) * TS],
                    enc_sb[:, c * TS : (c + 1) * TS],
                    mybir.ActivationFunctionType.Tanh,
                    bias=dec_sb[:, c * U + u : c * U + u + 1],
                )
            ps = psum_pool.tile([128, V], f32, tag="ps")
            for vh in range(VH):
                for c in range(CCH):
                    nc.tensor.matmul(
                        ps[:, vh * 512 : (vh + 1) * 512],
                        lhsT=lg[:, c * TS : (c + 1) * TS],
                        rhs=wt_sb[:, c * V + vh * 512 : c * V + vh * 512 + 512],
                        start=(c == 0),
                        stop=(c == CCH - 1),
                    )
            j = u % UG
            if j == 0:
                ob = out_pool.tile([128, UG * V], bf16, tag="ob")
            nc.vector.tensor_add(ob[:, j * V : (j + 1) * V], ps[:], bias_sb[:])
            if j == UG - 1:
                nc.sync.dma_start(
                    out[:, u - (UG - 1) : u + 1, :],
                    ob[:].rearrange("p (g v) -> p g v", g=UG),
                )

    nc.finalize()
    return nc


def _get_nc():
    if "nc" not in _CACHE:
        _CACHE["nc"] = _build()
    return _CACHE["nc"]


def kernel(**inputs):
    import ml_dtypes

    enc = np.asarray(inputs["enc_out"], dtype=np.float32)
    dec = np.asarray(inputs["dec_out"], dtype=np.float32)
    W = np.asarray(inputs["W"], dtype=np.float32)
    b = np.asarray(inputs["b"], dtype=np.float32)

    nc = _get_nc()

    wt_np = np.ascontiguousarray(W.T.astype(ml_dtypes.bfloat16))
    bias_np = np.ascontiguousarray(
        np.broadcast_to(b.astype(ml_dtypes.bfloat16), (128, V))
    )
    in_maps = []
    for k in range(NCORES):
        bb, t0 = k // 2, (k % 2) * TS
        in_maps.append(
            {
                "enc_t": np.ascontiguousarray(enc[bb, t0 : t0 + TS, :].T),
                "dec_t": np.ascontiguousarray(dec[bb].T),
                "wt": wt_np,
                "bias_rep": bias_np,
            }
        )

    from concourse.bass_utils import run_bass_kernel_spmd

    res = run_bass_kernel_spmd(nc, in_maps, list(range(NCORES)))
    _CACHE["last_result"] = res

    out = np.empty((B, T, U, V), np.float32)
    for k in range(NCORES):
        bb, t0 = k // 2, (k % 2) * TS
        out[bb, t0 : t0 + TS] = res.results[k]["out"].astype(np.float32)
    return out


# revision 9
# speedup vs baseline: 1.0404x; 1.0202x over previous
"""RNN-T Joiner kernel for Trainium2, data-parallel over (B, T) on 8 cores.

reference:
    logit = tanh(enc[:, :, None, :] + dec[:, None, :, :])   # (B,T,U,C)
    out   = einsum('btuc,vc->btuv', logit, W) + b           # (B,T,U,V)

Shapes (hardcoded): B=4, T=256, U=64, C=512, V=1024.

Sharding: core k handles b = k//2, t rows [ (k%2)*128, (k%2)*128+128 ).
W / bias replicated. No collectives.

Per-core device kernel (C on partitions for the logit):
  - logitT[c, t] = tanh(encT[c, t] + decT[c, u]) in bf16 -- scalar engine,
    fused per-partition bias add.
  - out[t, v] accumulated over 4 c-chunks of K=128 matmuls; both operands
    bf16 (full PE stream rate, fast weight load).
  - W / bias pre-cast to bf16 on host; output written bf16, upcast on host.
  - warmup matmuls on a memset tile run during the input DMA window so the
    PE HAM clock-gate is released before the real matmuls start.
  - bias add fused into the PSUM->SBUF eviction on DVE; 4 u-steps batched
    per output DMA (8KB contiguous per partition).
"""

import numpy as np

B, T, U, C, V = 4, 256, 64, 512, 1024
NCORES = 8
TS = 128  # t rows per core
CCH = C // 128  # 4 contraction chunks
VH = V // 512  # 2 psum-width chunks
UG = 4  # u-steps per output DMA batch
NWARM = 8  # warmup matmuls (N=512) to release the PE clock gate

_CACHE = {}


def _build():
    from contextlib import ExitStack

    import concourse.bacc as bacc
    import concourse.mybir as mybir
    import concourse.tile as tile

    dt = mybir.dt
    f32 = dt.float32
    bf16 = dt.bfloat16

    nc = bacc.Bacc("TRN2", target_bir_lowering=False, debug=False, num_devices=NCORES)
    # encdec: per-partition contiguous pack [128, c*TS | c*U] (enc then dec,
    # c-chunk-major within each) -> one DMA, one 3KB descriptor per partition.
    encdec = nc.declare_dram_parameter(
        "encdec", [128, CCH * (TS + U)], f32, isOutput=False
    )
    # wt: [128, c*V] bf16, DMA'd per c-chunk so chunk 0 lands early.
    wt = nc.declare_dram_parameter("wt", [128, CCH * V], bf16, isOutput=False)
    bias_rep = nc.declare_dram_parameter("bias_rep", [128, V], bf16, isOutput=False)
    out = nc.declare_dram_parameter("out", [TS, U, V], bf16, isOutput=True)

    with tile.TileContext(nc) as tc, ExitStack() as ctx:
        const = ctx.enter_context(tc.tile_pool(name="const", bufs=1))
        logit_pool = ctx.enter_context(tc.tile_pool(name="logit", bufs=6))
        psum_pool = ctx.enter_context(tc.tile_pool(name="psum", bufs=4, space="PSUM"))
        out_pool = ctx.enter_context(tc.tile_pool(name="out", bufs=3))

        wt_sb = const.tile([128, CCH * V], bf16, tag="wt")
        encdec_sb = const.tile([128, CCH * (TS + U)], f32, tag="encdec")
        bias_sb = const.tile([128, V], bf16, tag="bias")
        dummy = const.tile([128, 640], bf16, tag="dummy")
        DOFF = CCH * TS  # dec columns start here inside encdec_sb

        # Warmup source: memset on gpsimd (idle at startup), no DMA dep.
        nc.gpsimd.memset(dummy[:], 0.0)

        # Input DMAs: enc+dec packed as one contiguous-per-partition DMA on
        # the scalar ring (first: the tanh chain gates the first matmul);
        # W per-c-chunk + bias on the sync ring so chunk 0 lands early.
        nc.scalar.dma_start(encdec_sb[:], encdec[:])
        for c in range(CCH):
            nc.sync.dma_start(wt_sb[:, c * V : (c + 1) * V], wt[:, c * V : (c + 1) * V])
        nc.sync.dma_start(bias_sb[:], bias_rep[:])

        # Warmup matmuls: keep the PE busy while inputs stream in, so the
        # HAM clock-gate (4/8 cold -> 8/8 warm after ~3.4us of activity)
        # opens before the first real matmul.
        psw = psum_pool.tile([128, V], f32, tag="ps")
        for _ in range(NWARM):
            nc.tensor.matmul(
                psw[:, 0:512],
                lhsT=dummy[:, 0:128],
                rhs=dummy[:, 128:640],
                start=True,
                stop=True,
            )

        ob = None
        for u in range(U):
            lg = logit_pool.tile([128, CCH * TS], bf16, tag="lg")
            for c in range(CCH):
                nc.scalar.activation(
                    lg[:, c * TS : (c + 1) * TS],
                    encdec_sb[:, c * TS : (c + 1) * TS],
                    mybir.ActivationFunctionType.Tanh,
                    bias=encdec_sb[:, DOFF + c * U + u : DOFF + c * U + u + 1],
                )
            ps = psum_pool.tile([128, V], f32, tag="ps")
            for vh in range(VH):
                for c in range(CCH):
                    nc.tensor.matmul(
                        ps[:, vh * 512 : (vh + 1) * 512],
                        lhsT=lg[:, c * TS : (c + 1) * TS],
                        rhs=wt_sb[:, c * V + vh * 512 : c * V + vh * 512 + 512],
                        start=(c == 0),
                        stop=(c == CCH - 1),
                    )
            j = u % UG
            if j == 0:
                ob = out_pool.tile([128, UG * V], bf16, tag="ob")
            nc.vector.tensor_add(ob[:, j * V : (j + 1) * V], ps[:], bias_sb[:])
            if j == UG - 1:
                nc.sync.dma_start(
                    out[:, u - (UG - 1) : u + 1, :],
                    ob[:].rearrange("p (g v) -> p g v", g=UG),
                )

    nc.finalize()
    return nc


def _get_nc():
    if "nc" not in _CACHE:
        _CACHE["nc"] = _build()
    return _CACHE["nc"]


def kernel(**inputs):
    import ml_dtypes

    enc = np.asarray(inputs["enc_out"], dtype=np.float32)
    dec = np.asarray(inputs["dec_out"], dtype=np.float32)
    W = np.asarray(inputs["W"], dtype=np.float32)
    b = np.asarray(inputs["b"], dtype=np.float32)

    nc = _get_nc()

    # wt host layout: [p, c, v] = W.T[c*128+p, v] -> [128, CCH*V] bf16
    wt_np = np.ascontiguousarray(
        W.T.reshape(CCH, 128, V).transpose(1, 0, 2).reshape(128, CCH * V)
    ).astype(ml_dtypes.bfloat16)
    bias_np = np.ascontiguousarray(
        np.broadcast_to(b.astype(ml_dtypes.bfloat16), (128, V))
    )
    in_maps = []
    for k in range(NCORES):
        bb, t0 = k // 2, (k % 2) * TS
        # encdec pack: [p, (c-major enc t | c-major dec u)] f32
        enc_p = (
            enc[bb, t0 : t0 + TS, :].T.reshape(CCH, 128, TS).transpose(1, 0, 2)
        ).reshape(128, CCH * TS)
        dec_p = (dec[bb].T.reshape(CCH, 128, U).transpose(1, 0, 2)).reshape(
            128, CCH * U
        )
        in_maps.append(
            {
                "encdec": np.ascontiguousarray(
                    np.concatenate([enc_p, dec_p], axis=1)
                ),
                "wt": wt_np,
                "bias_rep": bias_np,
            }
        )

    from concourse.bass_utils import run_bass_kernel_spmd

    res = run_bass_kernel_spmd(nc, in_maps, list(range(NCORES)))
    _CACHE["last_result"] = res

    out = np.empty((B, T, U, V), np.float32)
    for k in range(NCORES):
        bb, t0 = k // 2, (k % 2) * TS
        out[bb, t0 : t0 + TS] = res.results[k]["out"].astype(np.float32)
    return out


# revision 12
# speedup vs baseline: 1.0586x; 1.0175x over previous
"""RNN-T Joiner kernel for Trainium2, data-parallel over (B, T) on 8 cores.

reference:
    logit = tanh(enc[:, :, None, :] + dec[:, None, :, :])   # (B,T,U,C)
    out   = einsum('btuc,vc->btuv', logit, W) + b           # (B,T,U,V)

Shapes (hardcoded): B=4, T=256, U=64, C=512, V=1024.

Sharding: core k handles b = k//2, t rows [ (k%2)*128, (k%2)*128+128 ).
W / bias replicated. No collectives.

Per-core device kernel (C on partitions for the logit):
  - logitT[c, t] = tanh(encT[c, t] + decT[c, u]) in bf16 -- scalar engine,
    fused per-partition bias add.
  - out[t, v] accumulated over 4 c-chunks of K=128 matmuls; both operands
    bf16 (full PE stream rate, fast weight load).
  - W / bias pre-cast to bf16 on host; output written bf16, upcast on host.
  - warmup matmuls on a memset tile run during the input DMA window so the
    PE HAM clock-gate is released before the real matmuls start.
  - bias add fused into the PSUM->SBUF eviction on DVE; 4 u-steps batched
    per output DMA (8KB contiguous per partition).
"""

import numpy as np

B, T, U, C, V = 4, 256, 64, 512, 1024
NCORES = 8
TS = 128  # t rows per core
CCH = C // 128  # 4 contraction chunks
VH = V // 512  # 2 psum-width chunks
UG = 4  # u-steps per output DMA batch
NWARM = 10  # warmup matmuls (N=512) to release the PE clock gate

_CACHE = {}


def _build():
    from contextlib import ExitStack

    import concourse.bacc as bacc
    import concourse.mybir as mybir
    import concourse.tile as tile

    dt = mybir.dt
    f32 = dt.float32
    bf16 = dt.bfloat16

    nc = bacc.Bacc("TRN2", target_bir_lowering=False, debug=False, num_devices=NCORES)
    # encdec: per-partition contiguous pack [128, c*TS | c*U] (enc then dec,
    # c-chunk-major within each) -> one DMA, one 3KB descriptor per partition.
    encdec = nc.declare_dram_parameter(
        "encdec", [128, CCH * (TS + U)], f32, isOutput=False
    )
    # wt: [128, c*V] bf16, DMA'd per c-chunk so chunk 0 lands early.
    wt = nc.declare_dram_parameter("wt", [128, CCH * V], bf16, isOutput=False)
    bias_rep = nc.declare_dram_parameter("bias_rep", [128, V], bf16, isOutput=False)
    out = nc.declare_dram_parameter("out", [TS, U, V], bf16, isOutput=True)

    with tile.TileContext(nc) as tc, ExitStack() as ctx:
        const = ctx.enter_context(tc.tile_pool(name="const", bufs=1))
        logit_pool = ctx.enter_context(tc.tile_pool(name="logit", bufs=6))
        psum_pool = ctx.enter_context(tc.tile_pool(name="psum", bufs=4, space="PSUM"))
        out_pool = ctx.enter_context(tc.tile_pool(name="out", bufs=3))

        wt_sb = const.tile([128, CCH * V], bf16, tag="wt")
        encdec_sb = const.tile([128, CCH * (TS + U)], f32, tag="encdec")
        bias_sb = const.tile([128, V], bf16, tag="bias")
        dummy = const.tile([128, 640], bf16, tag="dummy")
        DOFF = CCH * TS  # dec columns start here inside encdec_sb

        # Warmup source: memset on gpsimd (idle at startup), no DMA dep.
        nc.gpsimd.memset(dummy[:], 0.0)

        # Input DMAs all on the sync ring IN PRIORITY ORDER: the 16 DMA
        # engines drain the ring roughly FIFO, so encdec (which gates the
        # tanh chain and thus the first matmul) completes before the W
        # chunks start streaming; W chunk 0 still lands before matmul u=0.
        nc.sync.dma_start(encdec_sb[:], encdec[:])
        for c in range(CCH):
            nc.sync.dma_start(wt_sb[:, c * V : (c + 1) * V], wt[:, c * V : (c + 1) * V])
        nc.sync.dma_start(bias_sb[:], bias_rep[:])

        # Warmup matmuls: keep the PE busy while inputs stream in, so the
        # HAM clock-gate (4/8 cold -> 8/8 warm after ~3.4us of activity)
        # opens before the first real matmul.
        psw = psum_pool.tile([128, V], f32, tag="ps")
        for _ in range(NWARM):
            nc.tensor.matmul(
                psw[:, 0:512],
                lhsT=dummy[:, 0:128],
                rhs=dummy[:, 128:640],
                start=True,
                stop=True,
            )

        ob = None
        for u in range(U):
            lg = logit_pool.tile([128, CCH * TS], bf16, tag="lg")
            for c in range(CCH):
                nc.scalar.activation(
                    lg[:, c * TS : (c + 1) * TS],
                    encdec_sb[:, c * TS : (c + 1) * TS],
                    mybir.ActivationFunctionType.Tanh,
                    bias=encdec_sb[:, DOFF + c * U + u : DOFF + c * U + u + 1],
                )
            ps = psum_pool.tile([128, V], f32, tag="ps")
            for vh in range(VH):
                for c in range(CCH):
                    nc.tensor.matmul(
                        ps[:, vh * 512 : (vh + 1) * 512],
                        lhsT=lg[:, c * TS : (c + 1) * TS],
                        rhs=wt_sb[:, c * V + vh * 512 : c * V + vh * 512 + 512],
                        start=(c == 0),
                        stop=(c == CCH - 1),
                    )
            j = u % UG
            if j == 0:
                ob = out_pool.tile([128, UG * V], bf16, tag="ob")
            nc.vector.tensor_add(ob[:, j * V : (j + 1) * V], ps[:], bias_sb[:])
            if u >= U - UG:
                # last group: DMA per u-step so the final drain after the
                # last eviction is one 256KB transfer, not a 1MB batch
                nc.sync.dma_start(
                    out[:, u : u + 1, :],
                    ob[:, j * V : (j + 1) * V].rearrange("p (g v) -> p g v", g=1),
                )
            elif j == UG - 1:
                nc.sync.dma_start(
                    out[:, u - (UG - 1) : u + 1, :],
                    ob[:].rearrange("p (g v) -> p g v", g=UG),
                )

    nc.finalize()
    return nc


def _get_nc():
    if "nc" not in _CACHE:
        _CACHE["nc"] = _build()
    return _CACHE["nc"]


def kernel(**inputs):
    import ml_dtypes

    enc = np.asarray(inputs["enc_out"], dtype=np.float32)
    dec = np.asarray(inputs["dec_out"], dtype=np.float32)
    W = np.asarray(inputs["W"], dtype=np.float32)
    b = np.asarray(inputs["b"], dtype=np.float32)

    nc = _get_nc()

    # wt host layout: [p, c, v] = W.T[c*128+p, v] -> [128, CCH*V] bf16
    wt_np = np.ascontiguousarray(
        W.T.reshape(CCH, 128, V).transpose(1, 0, 2).reshape(128, CCH * V)
    ).astype(ml_dtypes.bfloat16)
    bias_np = np.ascontiguousarray(
        np.broadcast_to(b.astype(ml_dtypes.bfloat16), (128, V))
    )
    in_maps = []
    for k in range(NCORES):
        bb, t0 = k // 2, (k % 2) * TS
        # encdec pack: [p, (c-major enc t | c-major dec u)] f32
        enc_p = (
            enc[bb, t0 : t0 + TS, :].T.reshape(CCH, 128, TS).transpose(1, 0, 2)
        ).reshape(128, CCH * TS)
        dec_p = (dec[bb].T.reshape(CCH, 128, U).transpose(1, 0, 2)).reshape(
            128, CCH * U
        )
        in_maps.append(
            {
                "encdec": np.ascontiguousarray(
                    np.concatenate([enc_p, dec_p], axis=1)
                ),
                "wt": wt_np,
                "bias_rep": bias_np,
            }
        )

    from concourse.bass_utils import run_bass_kernel_spmd

    res = run_bass_kernel_spmd(nc, in_maps, list(range(NCORES)))
    _CACHE["last_result"] = res

    out = np.empty((B, T, U, V), np.float32)
    for k in range(NCORES):
        bb, t0 = k // 2, (k % 2) * TS
        out[bb, t0 : t0 + TS] = res.results[k]["out"].astype(np.float32)
    return out
